# revision 19
# baseline (speedup 1.0000x reference)
"""Trainium2 Bass kernel for a dense transformer block (attention + MLP).

Strategy: data-parallel over batch across 8 NeuronCores (48 batches each).
Per core, batches are processed in groups of 4 (512 tokens) so every dense
matmul has a 512-wide moving operand. Activations live transposed in SBUF
([feature, token]) so DRAM-layout weights serve directly as the stationary
matmul operand. Matmul operands are fp16 (full PE rate like bf16, but 10-bit
mantissa); accumulation is fp32 in PSUM and softmax statistics are fp32.
The input is transposed/cast on host; the fp16 output is transposed back and
upcast on host. Groups are software-pipelined: group i's attention chains are
interleaved with group i-1's MLP matmuls so the TensorE never idles on
softmax latency.
"""

from contextlib import ExitStack

import numpy as np

B, S, E, H, D, F = 384, 128, 512, 4, 128, 2048
NCORES = 8
BL = B // NCORES  # 48 batches per core
GB = 4  # batches per group
NTOK = GB * S  # 512 tokens per group
KE = E // 128  # 4
KF = F // 128  # 16

MMDT_NP = np.float16  # matmul operand dtype (fp16: full PE rate, 10-bit mantissa)

_cache: dict = {}


# --------------------------------------------------------------------------
# Workaround: the walrus build in this container accepts at most ONE
# sync-wait command per instruction, while Tile emits several. Hoist every
# extra wait onto its own preceding same-engine InstNoOp (engine queues are
# FIFO, so this is semantically identical).
def _fix_multiwaits(nc):
    import concourse.mybir as mybir

    n = 0
    for fn in nc.m.functions:
        for bb in fn.blocks:
            out = []
            changed = False
            for inst in bb.instructions:
                si = inst.sync_info
                if si is not None and len(si.on_wait) > 1:
                    waits = list(si.on_wait)
                    for w in waits[:-1]:
                        n += 1
                        out.append(
                            mybir.InstNoOp(
                                name=f"I-mwfix-{n}",
                                engine=inst.engine,
                                bass_nofuse=True,
                                sync_info=mybir.SyncInfo(on_wait=[w], on_update=[]),
                            )
                        )
                    inst.sync_info = mybir.SyncInfo(
                        on_wait=[waits[-1]], on_update=list(si.on_update)
                    )
                    changed = True
                out.append(inst)
            if changed:
                bb.instructions = out
    return n


def _build(ng, variant="full", repeat=1, cfg=None):
    """Build the per-core Bass program processing ng groups of 4 batches."""
    import concourse.bass as bass
    import concourse.mybir as mybir
    import concourse.tile as tile
    from concourse.masks import make_identity

    cfg = {
        **dict(
            big=2, att=2, atp=2, acc=2,
            xtp=2, qkp=2, vp=2, pp=4, zp=4, atsb=2, tmpp=3, xmp=2, hp=2, yp=3,
            gb=GB,
        ),
        **(cfg or {}),
    }
    gb = cfg["gb"]
    ntok = gb * S
    f32 = mybir.dt.float32
    bf16 = mybir.dt.float16
    AF = mybir.ActivationFunctionType
    ts = bass.ts

    ntok_total = BL * S

    nc = bass.Bass("TRN2", target_bir_lowering=False, debug=False)

    xt = nc.dram_tensor("xt", [E, ntok_total], bf16, kind="ExternalInput")
    wq_d = nc.dram_tensor("wq", [E, E], bf16, kind="ExternalInput")
    wk_d = nc.dram_tensor("wk", [E, E], bf16, kind="ExternalInput")
    wv_d = nc.dram_tensor("wv", [E, E], bf16, kind="ExternalInput")
    wo_d = nc.dram_tensor("wo", [E, E], bf16, kind="ExternalInput")
    w1_d = nc.dram_tensor("w1", [E, F], bf16, kind="ExternalInput")
    w2_d = nc.dram_tensor("w2", [F, E], bf16, kind="ExternalInput")
    bias_d = nc.dram_tensor("bias", [128, 32], f32, kind="ExternalInput")
    yt = nc.dram_tensor("yt", [E, ntok_total], bf16, kind="ExternalOutput")

    with tile.TileContext(nc) as tc, ExitStack() as ctx:
        singles = ctx.enter_context(tc.tile_pool(name="singles", bufs=1))

        def load_weight(name, dram, n_k, width):
            t = singles.tile([128, n_k, width], bf16, tag=f"w_{name}", name=f"w_{name}")
            nc.sync.dma_start(
                out=t, in_=dram[:, :].rearrange("(k p) w -> p k w", p=128)
            )
            return [t[:, k, :] for k in range(n_k)]

        wq_sb = load_weight("wq", wq_d, KE, E)
        wk_sb = load_weight("wk", wk_d, KE, E)
        wv_sb = load_weight("wv", wv_d, KE, E)
        wo_sb = load_weight("wo", wo_d, KE, E)
        w1_sb = load_weight("w1", w1_d, KE, F)
        w2_sb = load_weight("w2", w2_d, KF, E)

        bias_sb = singles.tile([128, 32], f32, tag="b_all", name="b_all")
        nc.sync.dma_start(out=bias_sb, in_=bias_d[:, :])
        bq_sb = bias_sb[:, 0:KE]
        bk_sb = bias_sb[:, KE : 2 * KE]
        bo_sb = bias_sb[:, 2 * KE : 3 * KE]
        b1_sb = bias_sb[:, 12 : 12 + KF]
        b2_sb = bias_sb[:, 28 : 28 + KE]

        ident = singles.tile([128, 128], bf16, tag="ident")
        make_identity(nc, ident)

        xtp = ctx.enter_context(tc.tile_pool(name="xtp", bufs=cfg["xtp"]))
        qkp = ctx.enter_context(tc.tile_pool(name="qkp", bufs=cfg["qkp"]))
        vp = ctx.enter_context(tc.tile_pool(name="vp", bufs=cfg["vp"]))
        pp = ctx.enter_context(tc.tile_pool(name="pp", bufs=cfg["pp"]))
        zp = ctx.enter_context(tc.tile_pool(name="zp", bufs=cfg["zp"]))
        atp = ctx.enter_context(tc.tile_pool(name="atp", bufs=cfg["atsb"]))
        tmpp = ctx.enter_context(tc.tile_pool(name="tmpp", bufs=cfg["tmpp"]))
        xmp = ctx.enter_context(tc.tile_pool(name="xmp", bufs=cfg["xmp"]))
        hp = ctx.enter_context(tc.tile_pool(name="hp", bufs=cfg["hp"]))
        yp = ctx.enter_context(tc.tile_pool(name="yp", bufs=cfg["yp"]))

        ps_big = ctx.enter_context(tc.tile_pool(name="ps_big", bufs=cfg["big"], space="PSUM"))
        ps_att = ctx.enter_context(tc.tile_pool(name="ps_att", bufs=cfg["att"], space="PSUM"))
        ps_atp = ctx.enter_context(tc.tile_pool(name="ps_atp", bufs=cfg["atp"], space="PSUM"))
        ps_acc = ctx.enter_context(tc.tile_pool(name="ps_acc", bufs=cfg["acc"], space="PSUM"))

        n_iters = ng * repeat
        st = {}  # per-iteration state

        def emit_load(i):
            g = i % ng
            c0 = g * ntok
            xt_t = xtp.tile([128, KE, ntok], bf16, tag="xt", name="xt_t")
            nc.sync.dma_start(
                out=xt_t,
                in_=xt[:, c0 : c0 + ntok].rearrange("(k p) t -> p k t", p=128),
            )
            s = st[i] = {}
            s["xt"] = [xt_t[:, k, :] for k in range(KE)]
            if variant == "mlponly":
                s["xm"] = s["xt"]

        def emit_qkv(i):
            s = st[i]
            xt_sb = s["xt"]
            if variant == "mlponly":
                return
            q_sb, k_sb = [], []
            for which, w_sb, b_sb, dst in (
                ("q", wq_sb, bq_sb, q_sb),
                ("k", wk_sb, bk_sb, k_sb),
            ):
                for h in range(H):
                    ps = ps_big.tile([128, ntok], f32, tag="big", name="qk_ps")
                    for k in range(KE):
                        nc.tensor.matmul(
                            ps,
                            w_sb[k][:, ts(h, 128)],
                            xt_sb[k],
                            start=(k == 0),
                            stop=(k == KE - 1),
                        )
                    t = qkp.tile([128, ntok], bf16, tag=f"{which}{h}", name=f"{which}{h}")
                    nc.vector.tensor_scalar_add(t, ps, b_sb[:, h : h + 1])
                    dst.append(t)
            v_sb = []
            for bi in range(gb):
                ps = ps_big.tile([128, E], f32, tag="big", name="v_ps")
                for k in range(KE):
                    nc.tensor.matmul(
                        ps,
                        xt_sb[k][:, ts(bi, 128)],
                        wv_sb[k],
                        start=(k == 0),
                        stop=(k == KE - 1),
                    )
                t = vp.tile([128, E], bf16, tag=f"v{bi}", name=f"v{bi}")
                nc.scalar.activation(out=t, in_=ps, func=AF.Copy)
                v_sb.append(t)
            s["q"], s["k"], s["v"] = q_sb, k_sb, v_sb
            at_t = atp.tile([128, H, ntok], bf16, tag="at", name="at_t")
            s["at_t"] = at_t
            s["at"] = q_sb if variant == "noattn" else [at_t[:, h, :] for h in range(H)]

        def emit_attn_batch(i, bi):
            s = st[i]
            q_sb, k_sb, v_sb, at_t = s["q"], s["k"], s["v"], s["at_t"]
            s_ps = ps_att.tile([128, H * 128], f32, tag="att", name="s_ps")
            for h in range(H):
                nc.tensor.matmul(
                    s_ps[:, ts(h, 128)],
                    q_sb[h][:, ts(bi, 128)],
                    k_sb[h][:, ts(bi, 128)],
                )
            # one exp over all heads (fp32: unnormalized exp needs range)
            p_sb = pp.tile([128, H * 128], f32, tag="p", name="p_sb")
            nc.scalar.activation(out=p_sb, in_=s_ps, func=AF.Exp)
            z_sb = zp.tile([128, H], f32, tag="z", name="z_sb")
            nc.vector.reduce_sum(
                z_sb, p_sb.rearrange("p (h s) -> p h s", h=H),
                axis=mybir.AxisListType.X,
            )
            rz_sb = zp.tile([128, H], f32, tag="rz", name="rz_sb")
            nc.vector.reciprocal(rz_sb, z_sb)
            pn_sb = pp.tile([128, H * 128], bf16, tag="pn", name="pn_sb")
            for h in range(H):
                nc.vector.tensor_scalar_mul(
                    pn_sb[:, ts(h, 128)], p_sb[:, ts(h, 128)], rz_sb[:, h : h + 1]
                )
            pt_sb = pp.tile([128, H * 128], bf16, tag="pt", name="pt_sb")
            if cfg.get("dma_transpose", False):
                for h in range(H):
                    nc.sync.dma_start(
                        out=pt_sb[:, ts(h, 128)], in_=pn_sb[:, ts(h, 128)],
                        transpose=True,
                    )
            else:
                pt_ps = ps_att.tile([128, H * 128], bf16, tag="att", name="pt_ps")
                for h in range(H):
                    nc.tensor.transpose(pt_ps[:, ts(h, 128)], pn_sb[:, ts(h, 128)], ident)
                nc.scalar.activation(out=pt_sb, in_=pt_ps, func=AF.Copy)
            at_ps = ps_atp.tile([128, H, 128], f32, tag="atp", name="at_ps")
            for h in range(H):
                nc.tensor.matmul(
                    at_ps[:, h, :], v_sb[bi][:, ts(h, 128)], pt_sb[:, ts(h, 128)]
                )
            nc.vector.tensor_copy(at_t[:, :, ts(bi, 128)], at_ps)

        def emit_outproj(i):
            if variant == "mlponly":
                return
            s = st[i]
            xm_sb = []
            for m in range(KE):
                ps = ps_big.tile([128, ntok], f32, tag="big", name="o_ps")
                for k in range(KE):
                    nc.tensor.matmul(
                        ps,
                        wo_sb[k][:, ts(m, 128)],
                        s["at"][k],
                        start=(k == 0),
                        stop=(k == KE - 1),
                    )
                tmp = tmpp.tile([128, ntok], bf16, tag="tmp", name="tmp")
                nc.scalar.activation(
                    out=tmp, in_=ps, func=AF.Identity, bias=bo_sb[:, m : m + 1]
                )
                xm = xmp.tile([128, ntok], bf16, tag=f"xm{m}", name=f"xm{m}")
                nc.gpsimd.tensor_add(xm, tmp, s["xt"][m])
                xm_sb.append(xm)
            s["xm"] = xm_sb

        def emit_mlp1_chunk(i, fs):
            s = st[i]
            h_sb = s.setdefault("h", [])
            for f in fs:
                ps = ps_big.tile([128, ntok], f32, tag="big", name="h_ps")
                for k in range(KE):
                    nc.tensor.matmul(
                        ps,
                        w1_sb[k][:, ts(f, 128)],
                        s["xm"][k],
                        start=(k == 0),
                        stop=(k == KE - 1),
                    )
                t = hp.tile([128, ntok], bf16, tag=f"h{f}", name=f"h{f}")
                if f % 2 == 0:
                    nc.scalar.activation(
                        out=t, in_=ps, func=AF.Relu, bias=b1_sb[:, f : f + 1]
                    )
                else:
                    nc.vector.tensor_scalar(
                        t, ps, b1_sb[:, f : f + 1], 0.0,
                        op0=mybir.AluOpType.add, op1=mybir.AluOpType.max,
                    )
                h_sb.append(t)

        def emit_mlp2_store(i):
            s = st[i]
            g = i % ng
            c0 = g * ntok
            h_sb = s["h"]
            yf = yp.tile([128, KE, ntok], bf16, tag="yf", name="yf")
            for m in range(KE):
                ps = ps_acc.tile([128, ntok], f32, tag="acc", name="acc_ps")
                for f in range(KF):
                    nc.tensor.matmul(
                        ps,
                        w2_sb[f][:, ts(m, 128)],
                        h_sb[f],
                        start=(f == 0),
                        stop=(f == KF - 1),
                    )
                tmpf = tmpp.tile([128, ntok], f32, tag="tmpf", name="tmpf")
                nc.scalar.activation(
                    out=tmpf, in_=ps, func=AF.Identity, bias=b2_sb[:, m : m + 1]
                )
                nc.gpsimd.tensor_add(yf[:, m, :], tmpf, s["xm"][m])
            nc.scalar.dma_start(
                out=yt[:, c0 : c0 + ntok].rearrange("(k p) t -> p k t", p=128),
                in_=yf,
            )
            del st[i]

        def emit_peonly(i):
            # PE-isolation variant: the exact matmul stream of a full group,
            # but fed from resident tiles with no inter-engine dataflow
            xts = st[i]["xt"]
            x128 = xts[0][:, 0:128]
            for h in range(2 * H):
                ps = ps_big.tile([128, ntok], f32, tag="big", name="qk_ps")
                for k in range(KE):
                    nc.tensor.matmul(ps, wq_sb[k][:, ts(h % H, 128)], xts[k],
                                     start=(k == 0), stop=(k == KE - 1))
            for bi in range(gb):
                ps = ps_big.tile([128, E], f32, tag="big", name="v_ps")
                for k in range(KE):
                    nc.tensor.matmul(ps, xts[k][:, ts(bi, 128)], wv_sb[k],
                                     start=(k == 0), stop=(k == KE - 1))
            for bi in range(gb):
                s_ps = ps_att.tile([128, H * 128], f32, tag="att", name="s_ps")
                for h in range(H):
                    nc.tensor.matmul(s_ps[:, ts(h, 128)], x128, x128)
                pt_ps = ps_att.tile([128, H * 128], bf16, tag="att", name="pt_ps")
                for h in range(H):
                    nc.tensor.transpose(pt_ps[:, ts(h, 128)], x128, ident)
                at_ps = ps_atp.tile([128, H, 128], f32, tag="atp", name="at_ps")
                for h in range(H):
                    nc.tensor.matmul(at_ps[:, h, :], x128, x128)
            for m in range(KE):
                ps = ps_big.tile([128, ntok], f32, tag="big", name="o_ps")
                for k in range(KE):
                    nc.tensor.matmul(ps, wo_sb[k][:, ts(m, 128)], xts[k],
                                     start=(k == 0), stop=(k == KE - 1))
            for f in range(KF):
                ps = ps_big.tile([128, ntok], f32, tag="big", name="h_ps")
                for k in range(KE):
                    nc.tensor.matmul(ps, w1_sb[k][:, ts(f, 128)], xts[k],
                                     start=(k == 0), stop=(k == KE - 1))
            for m in range(KE):
                ps = ps_acc.tile([128, ntok], f32, tag="acc", name="acc_ps")
                for f in range(KF):
                    nc.tensor.matmul(ps, w2_sb[f][:, ts(m, 128)], xts[0],
                                     start=(f == 0), stop=(f == KF - 1))

        if variant == "peonly":
            for i in range(n_iters):
                emit_load(i)
                emit_peonly(i)
                st.pop(i, None)
        # software pipeline: group i's attention interleaves with group
        # i-1's first MLP matmuls so the PE never idles on softmax latency
        for i in range(n_iters + 1 if variant != "peonly" else 0):
            if i < n_iters:
                emit_load(i)
                emit_qkv(i)
                if variant == "full":
                    for bi in range(gb):
                        emit_attn_batch(i, bi)
                        if i >= 1:
                            emit_mlp1_chunk(i - 1, range(KF * bi // gb, KF * (bi + 1) // gb))
                elif i >= 1:
                    emit_mlp1_chunk(i - 1, range(KF))
            elif i >= 1:
                emit_mlp1_chunk(i - 1, range(KF))
            if i >= 1:
                if len(st[i - 1].get("h", [])) < KF:
                    emit_mlp1_chunk(i - 1, range(len(st[i - 1].get("h", [])), KF))
                emit_mlp2_store(i - 1)
            if i < n_iters:
                emit_outproj(i)

    _fix_multiwaits(nc)
    return nc


def _get_program(ng, variant="full", repeat=1, cfg=None):
    key = ("nc", ng, variant, repeat, tuple(sorted((cfg or {}).items())))
    if key not in _cache:
        _cache[key] = _build(ng, variant, repeat, cfg)
    return _cache[key]


def kernel(
    x, wq, bq, wk, bk, wv, bv, wo, bo, w1, b1, w2, b2, _ng=BL // GB
):
    import os

    from concourse.bass_utils import run_bass_kernel_spmd

    # The NTFF trace hook module does not exist in this container; make sure
    # run_bass_kernel_spmd never takes the trace branch even if BASS_TRACE
    # is set in the environment.
    os.environ["BASS_NEVER_TRACE"] = "1"

    x = np.asarray(x, np.float32)
    to_bf = lambda a: np.ascontiguousarray(np.asarray(a, np.float32).astype(MMDT_NP))

    # host-side prep: shard + transpose + cast
    ntok_total = BL * S
    x_sh = x.reshape(NCORES, ntok_total, E)
    xts = [np.ascontiguousarray(x_sh[c].T).astype(MMDT_NP) for c in range(NCORES)]

    wq_b, wk_b, wv_b, wo_b, w1_b, w2_b = map(to_bf, (wq, wk, wv, wo, w1, w2))

    resh = lambda b, nk: np.asarray(b, np.float32).reshape(nk, 128).T
    # bv is folded into the output-projection bias: P rows sum to 1, so
    # attn@wo + bo == (P@v_nobias)@wo + (bv@wo + bo).
    bo_eff = (
        np.asarray(bv, np.float64) @ np.asarray(wo, np.float64)
        + np.asarray(bo, np.float64)
    ).astype(np.float32)
    bias_pack = np.zeros((128, 32), np.float32)
    bias_pack[:, 0:KE] = resh(bq, KE)
    bias_pack[:, KE : 2 * KE] = resh(bk, KE)
    bias_pack[:, 2 * KE : 3 * KE] = resh(bo_eff, KE)
    bias_pack[:, 12 : 12 + KF] = resh(b1, KF)
    bias_pack[:, 28 : 28 + KE] = resh(b2, KE)

    nc = _get_program(_ng)

    in_maps = []
    for c in range(NCORES):
        in_maps.append(
            {
                "xt": xts[c],
                "wq": wq_b,
                "wk": wk_b,
                "wv": wv_b,
                "wo": wo_b,
                "w1": w1_b,
                "w2": w2_b,
                "bias": bias_pack,
            }
        )

    res = run_bass_kernel_spmd(nc, in_maps, core_ids=list(range(NCORES)))
    _cache["last_result"] = res

    out = np.empty((NCORES, ntok_total, E), np.float32)
    for c in range(NCORES):
        out[c] = res.results[c]["yt"].T.astype(np.float32)
    return out.reshape(B, S, E)
